# revision 1
# baseline (speedup 1.0000x reference)
"""Trainium2 Bass kernel: causal multi-head attention with RoPE.

Problem: B=2, S=2048, D=1024, H=16 heads, hd=64, fp32.
Sharding: 4-way head-tensor-parallel x 2-way batch-data-parallel over 8 cores.
Each core handles one batch element and 4 heads (256 of the 1024 model dims),
computes its partial contribution to the output projection, and the host sums
the 4 partials per batch element.

Per-core pipeline (matmuls in fp32r ~ tf32-grade precision):
  - x arrives split as bf16 hi/lo pairs; the device transposes both with the
    DMA xbar (2-byte-only path) and reconstructs xT = hi + lo on DVE in f32r.
  - RoPE via double projection: host supplies row-permuted/negated weight
    copies W1/W2 so q_rot = (x@W1.T).T * cos + (x@W2.T).T * sin needs no
    cross-partition shuffles on device.
  - scores^T = k_rot^T.T @ q_rot^T per (head, q-chunk, k-block-pair) with 2x
    row-tiling (K=64) packing head pairs on the PE array; causal block skip.
  - exp on ACT over two k-blocks at a time (scale=1/8 fused); the two
    diagonal block-pairs are masked multiplicatively with precomputed pair
    masks (on GpSimd to keep DVE free).
  - P^T @ v via matmul with a ones-column appended to v (M=65) so the
    softmax denominator accumulates for free in PSUM row 64.
  - normalization deferred past the attention loop: 1/denom via ln/exp on
    ACT (batched so the ACT table set only switches twice), broadcast
    across partitions with a K=1 matmul, normalize O^T on DVE.
  - y_partial = O_norm @ Wo_slice.T accumulated over the 4 heads (K=64).
"""
import numpy as np
import ml_dtypes
from contextlib import ExitStack

import concourse.bass as bass
import concourse.tile as tile
from concourse.tile import add_dep_helper
from concourse import bacc, mybir
from concourse.bass_utils import run_bass_kernel_spmd

F32 = mybir.dt.float32
F32R = mybir.dt.float32r
BF16 = mybir.dt.bfloat16

B, S, D, H, HD = 2, 2048, 1024, 16, 64
NCORES = 8
TPG = 4            # head-TP degree (groups of 4 heads)
LH = H // TPG      # 4 local heads per core
LD = LH * HD       # 256 local dims
ROPE_BASE = 10000.0
QC = 512           # q chunk (matmul moving dim)
NQC = S // QC      # 4
NST = S // 128     # 16 s tiles
NDT = D // 128     # 8 d tiles

Exp = mybir.ActivationFunctionType.Exp
Log = mybir.ActivationFunctionType.Ln

_NC_CACHE = None


def _build():
    nc = bacc.Bacc("TRN2", target_bir_lowering=False, debug=False,
                   enable_asserts=True, num_devices=NCORES)

    xhi = nc.dram_tensor("xhi", [NDT, S, 128], BF16, kind="ExternalInput").ap()
    xlo = nc.dram_tensor("xlo", [NDT, S, 128], BF16, kind="ExternalInput").ap()
    w1qt = nc.dram_tensor("w1qt", [128, 2048], F32, kind="ExternalInput").ap()
    w2qt = nc.dram_tensor("w2qt", [128, 2048], F32, kind="ExternalInput").ap()
    w1kt = nc.dram_tensor("w1kt", [128, 2048], F32, kind="ExternalInput").ap()
    w2kt = nc.dram_tensor("w2kt", [128, 2048], F32, kind="ExternalInput").ap()
    wvt = nc.dram_tensor("wvt", [128, 2048], F32, kind="ExternalInput").ap()
    wot = nc.dram_tensor("wot", [2, 128, D], F32, kind="ExternalInput").ap()
    cos2 = nc.dram_tensor("cos2", [128, S], F32, kind="ExternalInput").ap()
    sin2 = nc.dram_tensor("sin2", [128, S], F32, kind="ExternalInput").ap()
    pairmask = nc.dram_tensor("pairmask", [2, 128, 1024], F32,
                              kind="ExternalInput").ap()
    y = nc.dram_tensor("y", [S, D], F32, kind="ExternalOutput").ap()

    with tile.TileContext(nc) as tc, ExitStack() as octx:
        # ---- persistent pools ----
        pers = octx.enter_context(tc.tile_pool(name="pers", bufs=1))
        qkp = octx.enter_context(tc.tile_pool(name="qkp", bufs=1))
        vp = octx.enter_context(tc.tile_pool(name="vp", bufs=1))

        ones_f = pers.tile([128, 64], F32, tag="ones_f")
        nc.vector.memset(ones_f[:], 1.0)
        ones64 = pers.tile([1, 64], F32R, tag="ones64")
        nc.vector.tensor_copy(ones64[:], ones_f[0:1, 0:64])

        # q/k rotated, per head-pair tile: rows = [hA:(y1 32|y2 32) | hB:...]
        qrot = [qkp.tile([128, S], F32R, tag=f"qrot{j}", name=f"qrot{j}")
                for j in range(2)]
        krot = [qkp.tile([128, S], F32R, tag=f"krot{j}", name=f"krot{j}")
                for j in range(2)]
        # v natural with per-head ones column: cols lh*65..lh*65+64 = v head
        # lh, col lh*65+64 = 1.0
        vsb = [vp.tile([128, 260], F32R, tag=f"v{st}", name=f"v{st}")
               for st in range(NST)]

        with ExitStack() as s1:
            s1p = s1.enter_context(tc.tile_pool(name="s1p", bufs=1))
            xtp = s1.enter_context(tc.tile_pool(name="xtp", bufs=1))
            ppps = s1.enter_context(tc.tile_pool(name="ppps", bufs=6, space="PSUM"))
            pvps = s1.enter_context(tc.tile_pool(name="pvps", bufs=2, space="PSUM"))

            xt = [xtp.tile([128, S], F32R, tag=f"xt{dt}", name=f"xt{dt}")
                  for dt in range(NDT)]

            last_tr = None
            for dt in range(NDT):
                ht = s1p.tile([128, S], BF16, tag="hstg", bufs=2, name="ht")
                nc.sync.dma_start_transpose(ht[:], xhi[dt])
                lt = s1p.tile([128, S], BF16, tag="lstg", bufs=2, name="lt")
                last_tr = nc.sync.dma_start_transpose(lt[:], xlo[dt])
                nc.vector.tensor_add(xt[dt][:], ht[:], lt[:])

            def after_tr(bi):
                add_dep_helper(bi.ins, last_tr.ins, sync=False,
                               reason="defer DMA past xbar transposes")
                return bi

            wtiles = []
            for nm, src_ap in (("wq1", w1qt), ("wq2", w2qt), ("wk1", w1kt),
                               ("wk2", w2kt), ("wv", wvt)):
                wr = s1p.tile([128, 2048], F32R, tag=nm, name=nm)
                after_tr(nc.gpsimd.dma_start(wr[:], src_ap))
                wtiles.append(wr)
            wq1, wq2, wk1, wk2, wv = wtiles

            cos_sb = s1p.tile([128, S], F32, tag="cos")
            after_tr(nc.sync.dma_start(cos_sb[:], cos2))
            sin_sb = s1p.tile([128, S], F32, tag="sin")
            after_tr(nc.sync.dma_start(sin_sb[:], sin2))

            # ---- phase P: q/k projections + RoPE (both head-pair tiles) ----
            for (w1, w2, rot) in ((wq1, wq2, qrot), (wk1, wk2, krot)):
                for jt in range(2):
                    for sc in range(NQC):
                        p1 = ppps.tile([128, QC], F32, tag="pp", name="p1")
                        for dt in range(NDT):
                            nc.tensor.matmul(
                                p1[:],
                                w1[:, dt * 256 + jt * 128: dt * 256 + jt * 128 + 128],
                                xt[dt][:, sc * QC:(sc + 1) * QC],
                                start=(dt == 0), stop=(dt == NDT - 1))
                        p2 = ppps.tile([128, QC], F32, tag="pp", name="p2")
                        for dt in range(NDT):
                            nc.tensor.matmul(
                                p2[:],
                                w2[:, dt * 256 + jt * 128: dt * 256 + jt * 128 + 128],
                                xt[dt][:, sc * QC:(sc + 1) * QC],
                                start=(dt == 0), stop=(dt == NDT - 1))
                        t1 = s1p.tile([128, QC], F32, tag="rt", bufs=3, name="t1")
                        nc.vector.tensor_mul(t1[:], p1[:],
                                             cos_sb[:, sc * QC:(sc + 1) * QC])
                        t2 = s1p.tile([128, QC], F32, tag="rt", bufs=3, name="t2")
                        nc.vector.tensor_mul(t2[:], p2[:],
                                             sin_sb[:, sc * QC:(sc + 1) * QC])
                        nc.vector.tensor_add(rot[jt][:, sc * QC:(sc + 1) * QC],
                                             t1[:], t2[:])

            # ---- phase V: v projection ----
            for st in range(NST):
                # ones cols at 64,129,194,259
                vdst = vsb[st].rearrange("p (h c) -> p h c", c=65)[:, :, 64:65]
                nc.vector.tensor_copy(vdst, ones_f[:, 0:4].rearrange(
                    "p (h c) -> p h c", c=1))
                pv = pvps.tile([128, 256], F32, tag="pv", name="pv")
                for dt in range(NDT):
                    nc.tensor.matmul(pv[:],
                                     xt[dt][:, st * 128:(st + 1) * 128],
                                     wv[:, dt * 256:(dt + 1) * 256],
                                     start=(dt == 0), stop=(dt == NDT - 1))
                # strided copy into per-head 65-col groups
                dst = vsb[st].rearrange("p (h c) -> p h c", c=65)[:, :, 0:64]
                src = pv.rearrange("p (h c) -> p h c", c=64)
                nc.scalar.copy(dst, src)

        # ---- attention-persistent tiles ----
        ap = octx.enter_context(tc.tile_pool(name="ap", bufs=1))
        masks = []
        for j in range(2):
            m = ap.tile([128, 1024], F32R, tag=f"mask{j}", name=f"m{j}")
            after_tr(nc.gpsimd.dma_start(m[:], pairmask[j]))
            masks.append(m)
        wo_sb = []
        for hp in range(2):
            w = ap.tile([128, D], F32R, tag=f"wo{hp}", name=f"wo{hp}")
            after_tr(nc.gpsimd.dma_start(w[:], wot[hp]))
            wo_sb.append(w)
        # O^T unnormalized, pair-stacked: rows 0:64 = head 2hp, 64:128 = 2hp+1
        otu = [ap.tile([128, S], F32, tag=f"otu{hp}", name=f"otu{hp}")
               for hp in range(2)]
        # softmax denominator and its reciprocal, per head, [1, S]
        den = [ap.tile([1, S], F32, tag=f"den{lh}", name=f"den{lh}")
               for lh in range(LH)]
        rsb = [ap.tile([1, S], F32R, tag=f"rsb{lh}", name=f"rsb{lh}")
               for lh in range(LH)]

        # ---- phase A: attention ----
        with ExitStack() as s2:
            s2p = s2.enter_context(tc.tile_pool(name="s2p", bufs=1))
            pss = s2.enter_context(tc.tile_pool(name="pss", bufs=3, space="PSUM"))
            pso = s2.enter_context(tc.tile_pool(name="pso", bufs=2, space="PSUM"))

            for hp in range(2):          # head pair = (2hp, 2hp+1)
                for qc in range(NQC):
                    npair = 2 * qc + 2
                    po = [pso.tile([128, QC], F32, tag="po", name="po")
                          for _ in range(2)]

                    def emit_pv(kp, pts, hp=hp, qc=qc, po=po):
                        kb0 = 2 * kp
                        for z in range(2):
                            lh = 2 * hp + z
                            for e in range(2):
                                kb = kb0 + e
                                nc.tensor.matmul(
                                    po[z][0:65, :],
                                    vsb[kb][:, lh * 65:lh * 65 + 65],
                                    pts[z][:, e * QC:(e + 1) * QC],
                                    start=(kb == 0),
                                    stop=(kb == 4 * qc + 3))

                    pend = []
                    for kp in range(npair):
                        kb0 = 2 * kp
                        pts = []
                        for z in range(2):   # z=0: rows 0:64, z=1: 64:128
                            r0 = 64 * z
                            ps_ = pss.tile([128, 1024], F32, tag="ps", name="ps_")
                            for e in range(2):
                                kb = kb0 + e
                                nc.tensor.matmul(
                                    ps_[:, e * QC:(e + 1) * QC],
                                    krot[hp][r0:r0 + 64, kb * 128:(kb + 1) * 128],
                                    qrot[hp][r0:r0 + 64, qc * QC:(qc + 1) * QC],
                                    start=True, stop=True, tile_position=(r0, 0))
                            pt = s2p.tile([128, 1024], F32R, tag="pt", bufs=6,
                                          name="pt")
                            nc.scalar.activation(pt[:], ps_[:], Exp, scale=0.125)
                            if kp >= npair - 2:  # diagonal block pair
                                j = 0 if kp == npair - 2 else 1
                                nc.vector.tensor_mul(pt[:], pt[:], masks[j][:])
                            pts.append(pt)
                        # software-pipeline: PV lags two rounds behind the
                        # scores so the PE never waits on exp/mask latency.
                        pend.append((kp, pts))
                        if len(pend) > 2:
                            emit_pv(*pend.pop(0))
                    for kp_, pts_ in pend:
                        emit_pv(kp_, pts_)
                    for z in range(2):
                        lh = 2 * hp + z
                        nc.vector.tensor_copy(
                            otu[hp][64 * z:64 * z + 64, qc * QC:(qc + 1) * QC],
                            po[z][0:64, :])
                        nc.vector.tensor_copy(
                            den[lh][0:1, qc * QC:(qc + 1) * QC],
                            po[z][64:65, :])
                    if hp == 1 and qc == 2:
                        emit_recips(1, 0, 3 * QC)
                    if hp == 1 and qc == 3:
                        emit_recips(1, 3 * QC, S)
                # denominator reciprocals (ln+exp batches). hp0's run on ACT
                # during hp1's attention; hp1's are split so most of the work
                # overlaps its own qc3 instead of stalling phase Y.
                def emit_recips(hp, c0, c1):
                    for z in range(2):
                        lh = 2 * hp + z
                        lnd = s2p.tile([1, S], F32, tag="lnd", bufs=2,
                                       name="lnd")
                        nc.scalar.activation(lnd[0:1, c0:c1],
                                             den[lh][0:1, c0:c1], Log)
                        nc.scalar.activation(rsb[lh][0:1, c0:c1],
                                             lnd[0:1, c0:c1], Exp, scale=-1.0)
                if hp == 0:
                    emit_recips(0, 0, S)

        # ---- phases N+Y interleaved: normalize + output projection ----
        with ExitStack() as s3:
            s3p = s3.enter_context(tc.tile_pool(name="s3p", bufs=1))
            otnp = s3.enter_context(tc.tile_pool(name="otnp", bufs=1))
            psb = s3.enter_context(tc.tile_pool(name="psb", bufs=2, space="PSUM"))
            psy = s3.enter_context(tc.tile_pool(name="psy", bufs=3, space="PSUM"))

            otn = [otnp.tile([128, S], F32R, tag=f"otn{hp}", name=f"otn{hp}")
                   for hp in range(2)]
            for qc in range(NQC):
                for hp in range(2):
                    pbA = psb.tile([128, QC], F32, tag="pb", name="pbA")
                    nc.tensor.matmul(pbA[0:64, :], ones64[:],
                                     rsb[2 * hp][0:1, qc * QC:(qc + 1) * QC],
                                     start=True, stop=True)
                    pbB = psb.tile([128, QC], F32, tag="pb", name="pbB")
                    nc.tensor.matmul(pbB[0:64, :], ones64[:],
                                     rsb[2 * hp + 1][0:1, qc * QC:(qc + 1) * QC],
                                     start=True, stop=True)
                    nc.vector.tensor_mul(
                        otn[hp][0:64, qc * QC:(qc + 1) * QC],
                        pbA[0:64, :], otu[hp][0:64, qc * QC:(qc + 1) * QC])
                    nc.vector.tensor_mul(
                        otn[hp][64:128, qc * QC:(qc + 1) * QC],
                        pbB[0:64, :], otu[hp][64:128, qc * QC:(qc + 1) * QC])
                for st in range(4 * qc, 4 * qc + 4):
                    ysb = s3p.tile([128, D], F32, tag="ysb", bufs=3, name="ysb")
                    for mc in range(2):
                        py = psy.tile([128, 512], F32, tag="py", name="py")
                        for hp in range(2):
                            nc.tensor.matmul(
                                py[:],
                                otn[hp][:, st * 128:(st + 1) * 128],
                                wo_sb[hp][:, mc * 512:(mc + 1) * 512],
                                start=(hp == 0), stop=(hp == 1))
                        dst = ysb[:, mc * 512:(mc + 1) * 512]
                        if mc == 0:
                            nc.scalar.copy(dst, py[:])
                        else:
                            nc.vector.tensor_copy(dst, py[:])
                    nc.sync.dma_start(y[st * 128:(st + 1) * 128, :], ysb[:])

    nc.compile()
    return nc


def _get_nc():
    global _NC_CACHE
    if _NC_CACHE is None:
        _NC_CACHE = _build()
    return _NC_CACHE


def _host_prep(x, Wq, Wk, Wv, Wo):
    """Build the 8 per-core input maps."""
    x = np.asarray(x, dtype=np.float32)
    Wq, Wk, Wv, Wo = (np.asarray(w, dtype=np.float32) for w in (Wq, Wk, Wv, Wo))

    def tile128(wt):  # [1024, 256] -> [128, 2048] with d-tiles along free dim
        return np.ascontiguousarray(
            wt.reshape(NDT, 128, LD).transpose(1, 0, 2).reshape(128, NDT * LD))

    def perm_pair(W, g):
        blocks1, blocks2 = [], []
        for lh in range(LH):
            gh = g * LH + lh
            O = W[gh * HD:(gh + 1) * HD]          # [64, 1024]
            ev, od = O[0::2], O[1::2]
            blocks1.append(np.concatenate([ev, od], axis=0))
            blocks2.append(np.concatenate([-od, ev], axis=0))
        W1 = np.concatenate(blocks1, axis=0)      # [256, 1024]
        W2 = np.concatenate(blocks2, axis=0)
        return tile128(W1.T), tile128(W2.T)

    t = np.arange(32, dtype=np.float64)
    theta = 1.0 / (ROPE_BASE ** (2.0 * t / HD))
    ang = np.arange(S, dtype=np.float64)[:, None] * theta[None, :]  # [S, 32]
    c32 = np.cos(ang).T.astype(np.float32)        # [32, S]
    s32 = np.sin(ang).T.astype(np.float32)
    cos2 = np.ascontiguousarray(np.tile(c32, (4, 1)))   # [128, S]
    sin2 = np.ascontiguousarray(np.tile(s32, (4, 1)))

    # pair masks for the two diagonal block-pairs of each (z, qc):
    # mask[j][p, u]: j=0 covers relative blocks (0, 1), j=1 covers (2, 3).
    p = np.arange(128)[:, None]
    u = np.arange(1024)[None, :]
    pm = np.zeros((2, 128, 1024), dtype=np.float32)
    for j in range(2):
        off = 256 * j
        pm[j] = np.where(u < 512, p <= u - off, p <= u - 640 - off)
    pairmask = np.ascontiguousarray(pm)

    bf = ml_dtypes.bfloat16
    per_b = []
    for b in range(B):
        xhi = x[b].astype(bf)
        xlo = (x[b] - xhi.astype(np.float32)).astype(bf)
        xhi = xhi.reshape(S, NDT, 128).transpose(1, 0, 2)
        xlo = xlo.reshape(S, NDT, 128).transpose(1, 0, 2)
        per_b.append((np.ascontiguousarray(xhi), np.ascontiguousarray(xlo)))

    per_g = []
    for g in range(TPG):
        w1q, w2q = perm_pair(Wq, g)
        w1k, w2k = perm_pair(Wk, g)
        wvt_ = tile128(Wv[g * LD:(g + 1) * LD].T)
        wot_ = np.ascontiguousarray(
            Wo[:, g * LD:(g + 1) * LD].T.reshape(2, 128, D))
        per_g.append((w1q, w2q, w1k, w2k, wvt_, wot_))

    in_maps = []
    for c in range(NCORES):
        b, g = divmod(c, TPG)
        w1q, w2q, w1k, w2k, wvt_, wot_ = per_g[g]
        in_maps.append({
            "xhi": per_b[b][0], "xlo": per_b[b][1],
            "w1qt": w1q, "w2qt": w2q, "w1kt": w1k, "w2kt": w2k,
            "wvt": wvt_, "wot": wot_,
            "cos2": cos2, "sin2": sin2,
            "pairmask": pairmask,
        })
    return in_maps


def run(inputs, trace=False):
    """Run on all 8 cores; returns (y_full, BassKernelResults)."""
    x = inputs["x"]
    in_maps = _host_prep(x, inputs["Wq"], inputs["Wk"], inputs["Wv"],
                         inputs["Wo"])
    nc = _get_nc()
    kw = {}
    if trace:
        kw = dict(trace=True, trace_cores=[0])
    res = run_bass_kernel_spmd(nc, in_maps, core_ids=list(range(NCORES)), **kw)
    y = np.zeros((B, S, D), dtype=np.float32)
    for c in range(NCORES):
        y[c // TPG] += res.results[c]["y"]
    return y, res


def kernel(x, Wq, Wk, Wv, Wo, n_heads):
    assert int(n_heads) == H
    y, _ = run({"x": x, "Wq": Wq, "Wk": Wk, "Wv": Wv, "Wo": Wo})
    return y



# revision 15
# speedup vs baseline: 1.3808x; 1.3808x over previous
"""Trainium2 Bass kernel: causal multi-head attention with RoPE.

Problem: B=2, S=2048, D=1024, H=16 heads, hd=64, fp32.
Sharding: 4-way head-tensor-parallel x 2-way batch-data-parallel over 8 cores.
Each core handles one batch element and 4 heads (256 of the 1024 model dims),
computes its partial contribution to the output projection, and the host sums
the 4 partials per batch element.

Per-core pipeline (bf16 matmul operands, fp32 PSUM accumulation):
  - x arrives as one bf16 tensor; DMA-xbar transposes it in per-512-column
    chunks so the first projection matmul starts ~6us in, with weight DMAs
    running concurrently on other queues.
  - Q/K use a single projection each. The host permutes W rows so each head's
    dims land as [even-dims (32) | odd-dims (32)]; RoPE is then 5 DVE ops per
    chunk using partition-offset operands (T1=x*cos, T2=x*sin, then
    y1 = T1[p] - T2[p+32] / y2 = T2[p-32] + T1[p] written via 3 region ops).
  - scores^T = k_rot^T.T @ q_rot^T per (head-pair, q-chunk, k-block-pair) with
    2x row-tiling (K=64); causal block skip plus column-suffix restriction on
    the 4 diagonal blocks (only q >= k-block-start columns are computed).
  - exp on ACT (scale=1/8 fused) over alive column ranges only; the four
    per-block [128,128] causal triangles are masked multiplicatively on DVE.
  - P^T @ v via matmul with a ones-column appended to v (M=65) so the softmax
    denominator accumulates free in PSUM row 64.
  - 1/den via DVE reciprocal_approx_fast (no ACT table switches); broadcast
    across partitions with K=1 matmuls; normalize PV output straight out of
    PSUM on DVE.
  - y chunk = O_norm @ Wo_slice.T per q-chunk, DMA'd out immediately
    (qc-outer loop: attention, normalize, output projection all interleave).
"""
import numpy as np
import ml_dtypes
from contextlib import ExitStack

import concourse.bass as bass
import concourse.tile as tile
from concourse import bacc, mybir
from concourse.bass_utils import run_bass_kernel_spmd

F32 = mybir.dt.float32
F32R = mybir.dt.float32r
BF16 = mybir.dt.bfloat16

B, S, D, H, HD = 2, 2048, 1024, 16, 64
NCORES = 8
TPG = 4            # head-TP degree (groups of 4 heads)
LH = H // TPG      # 4 local heads per core
LD = LH * HD       # 256 local dims
ROPE_BASE = 10000.0
QC = 512           # q chunk (matmul moving dim)
NQC = S // QC      # 4
NST = S // 128     # 16 s tiles
NDT = D // 128     # 8 d tiles

Exp = mybir.ActivationFunctionType.Exp

_NC_CACHE = None
DEBUG = False


def _build():
    nc = bacc.Bacc("TRN2", target_bir_lowering=False, debug=False,
                   enable_asserts=True, num_devices=NCORES)

    xbf = nc.dram_tensor("xbf", [NDT, S, 128], BF16, kind="ExternalInput").ap()
    wqt = nc.dram_tensor("wqt", [128, 2048], BF16, kind="ExternalInput").ap()
    wkt = nc.dram_tensor("wkt", [128, 2048], BF16, kind="ExternalInput").ap()
    wvt = nc.dram_tensor("wvt", [128, 2048], BF16, kind="ExternalInput").ap()
    wot = nc.dram_tensor("wot", [2, 128, D], BF16, kind="ExternalInput").ap()
    cosd = nc.dram_tensor("cosd", [128, S], BF16, kind="ExternalInput").ap()
    sind = nc.dram_tensor("sind", [128, S], BF16, kind="ExternalInput").ap()
    triud = nc.dram_tensor("triud", [128, 128], BF16,
                           kind="ExternalInput").ap()
    y = nc.dram_tensor("y", [S, D], F32, kind="ExternalOutput").ap()
    dbg = (nc.dram_tensor("dbg", [1024, 1024], F32, kind="ExternalOutput").ap()
           if DEBUG else None)

    with tile.TileContext(nc) as tc, ExitStack() as octx:
        # ---- persistent pools ----
        pers = octx.enter_context(tc.tile_pool(name="pers", bufs=1))
        qkp = octx.enter_context(tc.tile_pool(name="qkp", bufs=1))
        vp = octx.enter_context(tc.tile_pool(name="vp", bufs=1))

        # q/k rotated, per head-pair tile: rows = [hA: y1(32)|y2(32) | hB:...]
        qrot = [qkp.tile([128, S], BF16, tag=f"qrot{j}", name=f"qrot{j}")
                for j in range(2)]
        krot = [qkp.tile([128, S], BF16, tag=f"krot{j}", name=f"krot{j}")
                for j in range(2)]
        # v natural with per-head ones column: cols lh*65..lh*65+64 = v head
        # lh, col lh*65+64 = 1.0
        vsb = [vp.tile([128, 260], BF16, tag=f"v{st}", name=f"v{st}")
               for st in range(NST)]

        # weights / rope tables / mask, DMA'd up front on side queues
        wq = pers.tile([128, 2048], BF16, tag="wq", name="wq")
        nc.gpsimd.dma_start(wq[:], wqt)
        wk = pers.tile([128, 2048], BF16, tag="wk", name="wk")
        nc.gpsimd.dma_start(wk[:], wkt)
        wv = pers.tile([128, 2048], BF16, tag="wv", name="wv")
        nc.gpsimd.dma_start(wv[:], wvt)
        wo_sb = []
        for hp in range(2):
            w = pers.tile([128, D], BF16, tag=f"wo{hp}", name=f"wo{hp}")
            nc.gpsimd.dma_start(w[:], wot[hp])
            wo_sb.append(w)
        cos_sb = pers.tile([128, S], BF16, tag="cos")
        nc.scalar.dma_start(cos_sb[:], cosd)
        sin_sb = pers.tile([128, S], BF16, tag="sin")
        nc.scalar.dma_start(sin_sb[:], sind)
        triu = pers.tile([128, 128], BF16, tag="triu")
        nc.scalar.dma_start(triu[:], triud)

        xtp = octx.enter_context(tc.tile_pool(name="xtp", bufs=1))
        xt = [xtp.tile([128, S], BF16, tag=f"xt{dt}", name=f"xt{dt}")
              for dt in range(NDT)]

        # ---- phase P: projections + RoPE, chunked by 512-col s-chunks ----
        with ExitStack() as s1:
            s1p = s1.enter_context(tc.tile_pool(name="s1p", bufs=1))
            pqs = s1.enter_context(tc.tile_pool(name="pqs", bufs=3,
                                                space="PSUM"))
            pvs = s1.enter_context(tc.tile_pool(name="pvs", bufs=2,
                                                space="PSUM"))

            for st in range(NST):
                vdst = vsb[st].rearrange("p (h c) -> p h c", c=65)[:, :, 64:65]
                nc.vector.memset(vdst, 1.0)

            for sc in range(NQC):
                c0, c1 = sc * QC, (sc + 1) * QC
                for dt in range(NDT):
                    nc.sync.dma_start_transpose(
                        xt[dt][:, c0:c1], xbf[dt][c0:c1, :])

                # q/k projection + RoPE for this s-chunk
                for (w, rot) in ((wq, qrot), (wk, krot)):
                    for jt in range(2):
                        pq = pqs.tile([128, QC], F32, tag="pq", name="pq")
                        for dt in range(NDT):
                            nc.tensor.matmul(
                                pq[:],
                                w[:, dt * 256 + jt * 128:
                                  dt * 256 + jt * 128 + 128],
                                xt[dt][:, c0:c1],
                                start=(dt == 0), stop=(dt == NDT - 1))
                        xb = s1p.tile([128, QC], BF16, tag="xb", bufs=3,
                                      name="xb")
                        nc.scalar.copy(xb[:], pq[:])
                        t1 = s1p.tile([128, QC], BF16, tag="t1", bufs=2,
                                      name="t1")
                        nc.vector.tensor_mul(t1[:], xb[:], cos_sb[:, c0:c1])
                        # t2s[p] = x[swap32(p)] * sneg[swap32(p)] where
                        # swap32 flips adjacent 32-row blocks and sneg has
                        # -sin on the x2 rows; then rot = t1 + t2s gives
                        # y1 = x1*c - x2*s, y2 = x2*c + x1*s in one add.
                        # (DVE in-operands must share a base partition;
                        # only the out base may shift.)
                        t2 = s1p.tile([128, QC], BF16, tag="t2", bufs=2,
                                      name="t2")
                        for blk in range(4):
                            a, b = blk * 32, (blk ^ 1) * 32
                            nc.vector.tensor_mul(t2[a:a + 32, :],
                                                 xb[b:b + 32, :],
                                                 sin_sb[b:b + 32, c0:c1])
                        nc.vector.tensor_add(rot[jt][:, c0:c1], t1[:], t2[:])
                        if DEBUG and w is wq and jt == 0 and sc == 0:
                            nc.gpsimd.dma_start(dbg[260:388, 0:512], xb[:])
                            nc.gpsimd.dma_start(dbg[644:772, 0:512],
                                                rot[0][:, 0:512])

                # v projection for the 4 s-tiles of this chunk
                for st in range(4 * sc, 4 * sc + 4):
                    pv = pvs.tile([128, 256], F32, tag="pv", name="pv")
                    for dt in range(NDT):
                        nc.tensor.matmul(
                            pv[:],
                            xt[dt][:, st * 128:(st + 1) * 128],
                            wv[:, dt * 256:(dt + 1) * 256],
                            start=(dt == 0), stop=(dt == NDT - 1))
                    dst = vsb[st].rearrange("p (h c) -> p h c",
                                            c=65)[:, :, 0:64]
                    src = pv.rearrange("p (h c) -> p h c", c=64)
                    nc.scalar.copy(dst, src)
                    if DEBUG and st == 0:
                        nc.gpsimd.dma_start(dbg[388:516, 0:260],
                                            vsb[0][:, 0:260])

        # ---- attention + normalize + output projection, qc-outer ----
        with ExitStack() as s2:
            s2p = s2.enter_context(tc.tile_pool(name="s2p", bufs=1))
            pss = s2.enter_context(tc.tile_pool(name="pss", bufs=2,
                                                space="PSUM"))
            pso = s2.enter_context(tc.tile_pool(name="pso", bufs=2,
                                                space="PSUM"))
            psy = s2.enter_context(tc.tile_pool(name="psy", bufs=2,
                                                space="PSUM"))

            pend_y = [None]

            def emit_y(qc, otn):
                for j in range(4):
                    st = 4 * qc + j
                    ysb = s2p.tile([128, D], F32, tag="ysb", bufs=3,
                                   name="ysb")
                    for mc in range(2):
                        py = psy.tile([128, 512], F32, tag="py", name="py")
                        for hp in range(2):
                            nc.tensor.matmul(
                                py[:],
                                otn[hp][:, j * 128:(j + 1) * 128],
                                wo_sb[hp][:, mc * 512:(mc + 1) * 512],
                                start=(hp == 0), stop=(hp == 1))
                        dst = ysb[:, mc * 512:(mc + 1) * 512]
                        if mc == 0:
                            nc.scalar.copy(dst, py[:])
                        else:
                            nc.vector.tensor_copy(dst, py[:])
                    nc.sync.dma_start(y[st * 128:(st + 1) * 128, :], ysb[:])

            for qc in range(NQC):
                npair = 2 * qc + 2
                otn = [s2p.tile([128, QC], BF16, tag=f"otn{hp}", bufs=2,
                                name=f"otn{hp}") for hp in range(2)]
                for hp in range(2):          # head pair = (2hp, 2hp+1)
                    po = [pso.tile([128, QC], F32, tag="po", name="po")
                          for _ in range(2)]

                    def emit_pv(kp, pts, mlist, hp=hp, qc=qc, po=po):
                        for z in range(2):
                            lh = 2 * hp + z
                            for e in range(2):
                                kb = 2 * kp + e
                                m = mlist[e]
                                nc.tensor.matmul(
                                    po[z][0:65, m:QC],
                                    vsb[kb][:, lh * 65:lh * 65 + 65],
                                    pts[z][:, e * QC + m:(e + 1) * QC],
                                    start=(kb == 0),
                                    stop=(kb == 4 * qc + 3))

                    pend = []
                    for kp in range(npair):
                        diag = kp >= npair - 2
                        # alive-column start within the qc block, per e
                        mlist = []
                        for e in range(2):
                            kb = 2 * kp + e
                            r = kb - 4 * qc
                            mlist.append(128 * r if diag else 0)
                        pts = []
                        for z in range(2):   # z=0: rows 0:64, z=1: 64:128
                            r0 = 64 * z
                            ps_ = pss.tile([128, 1024], F32, tag="ps",
                                           name="ps_")
                            for e in range(2):
                                kb = 2 * kp + e
                                m = mlist[e]
                                nc.tensor.matmul(
                                    ps_[:, e * QC + m:(e + 1) * QC],
                                    krot[hp][r0:r0 + 64,
                                             kb * 128:(kb + 1) * 128],
                                    qrot[hp][r0:r0 + 64,
                                             qc * QC + m:(qc + 1) * QC],
                                    start=True, stop=True,
                                    tile_position=(r0, 0))
                            pt = s2p.tile([128, 1024], BF16, tag="pt",
                                          bufs=6, name="pt")
                            if not diag:
                                nc.scalar.activation(pt[:], ps_[:], Exp,
                                                     scale=0.125)
                            else:
                                for e in range(2):
                                    m = mlist[e]
                                    nc.scalar.activation(
                                        pt[:, e * QC + m:(e + 1) * QC],
                                        ps_[:, e * QC + m:(e + 1) * QC],
                                        Exp, scale=0.125)
                                    # causal triangle on the first 128 alive
                                    # columns of each diagonal block
                                    nc.vector.tensor_mul(
                                        pt[:, e * QC + m:e * QC + m + 128],
                                        pt[:, e * QC + m:e * QC + m + 128],
                                        triu[:])
                            if (DEBUG and qc == 0 and hp == 0 and kp == 0
                                    and z == 0):
                                nc.gpsimd.dma_start(dbg[516:644, 0:1024],
                                                    pt[:])
                            pts.append(pt)
                        # software-pipeline: PV lags two rounds behind the
                        # scores so the PE never waits on exp/mask latency.
                        pend.append((kp, pts, mlist))
                        if len(pend) > 2:
                            emit_pv(*pend.pop(0))
                        if kp == 0 and hp == 0 and pend_y[0] is not None:
                            # previous qc's output projection slots in here,
                            # after this qc's first scores feed ACT
                            emit_y(qc - 1, pend_y[0])
                            pend_y[0] = None
                    for args in pend:
                        emit_pv(*args)

                    # softmax denominators -> reciprocals -> partition
                    # broadcast -> normalized O^T (bf16) for this head pair.
                    # (partition_broadcast requires an out AP based at
                    # partition 0, hence per-z [64, QC] tiles)
                    for z in range(2):
                        dsb = s2p.tile([1, QC], F32, tag="dsb", bufs=2,
                                       name="dsb")
                        nc.vector.tensor_copy(dsb[:], po[z][64:65, :])
                        r = s2p.tile([1, QC], F32, tag="rsb", bufs=4,
                                     name="rsb")
                        nc.vector.reciprocal_approx_fast(
                            out=r[:], in_=dsb[:])
                        pbs = s2p.tile([64, QC], F32, tag="pbs", bufs=4,
                                       name="pbs")
                        nc.gpsimd.partition_broadcast(pbs[:], r[:])
                        nc.vector.tensor_mul(otn[hp][64 * z:64 * z + 64, :],
                                             po[z][0:64, :], pbs[:])
                        if DEBUG and qc == 0 and hp == 0:
                            nc.sync.dma_start(dbg[z:z + 1, 0:512], r[:])
                            nc.sync.dma_start(dbg[4 + 64 * z:4 + 64 * z + 64,
                                                  0:512], pbs[:])
                    if DEBUG and qc == 0 and hp == 0:
                        nc.gpsimd.dma_start(dbg[132:260, 0:512],
                                            otn[0][:])
                pend_y[0] = otn
            emit_y(NQC - 1, pend_y[0])

    nc.compile()
    return nc


def _get_nc():
    global _NC_CACHE
    if _NC_CACHE is None:
        _NC_CACHE = _build()
    return _NC_CACHE


def _host_prep(x, Wq, Wk, Wv, Wo):
    """Build the 8 per-core input maps."""
    bf = ml_dtypes.bfloat16
    x = np.asarray(x, dtype=np.float32)
    Wq, Wk, Wv, Wo = (np.asarray(w, dtype=np.float32) for w in (Wq, Wk, Wv, Wo))

    def tile128(wt):  # [1024, 256] -> [128, 2048] with d-tiles along free dim
        return np.ascontiguousarray(
            wt.reshape(NDT, 128, LD).transpose(1, 0, 2).reshape(
                128, NDT * LD).astype(bf))

    def perm_qk(W, g):
        # rows per jt tile: [h0 evens(32) | h0 odds(32) | h1 evens | h1 odds]
        blocks = []
        for lh in range(LH):
            gh = g * LH + lh
            O = W[gh * HD:(gh + 1) * HD]          # [64, 1024]
            blocks.append(O[0::2])
            blocks.append(O[1::2])
        Wp = np.concatenate(blocks, axis=0)       # [256, 1024]
        return tile128(Wp.T)

    t = np.arange(32, dtype=np.float64)
    theta = 1.0 / (ROPE_BASE ** (2.0 * t / HD))
    ang = np.arange(S, dtype=np.float64)[:, None] * theta[None, :]  # [S, 32]
    c32 = np.cos(ang).T.astype(np.float32)        # [32, S]
    s32 = np.sin(ang).T.astype(np.float32)
    cosd = np.ascontiguousarray(np.tile(c32, (4, 1)).astype(bf))  # [128, S]
    sneg = np.tile(s32, (4, 1))
    sneg[32:64] *= -1.0   # x2 rows carry -sin so rot = x*cos + swap(x*sneg)
    sneg[96:128] *= -1.0
    sind = np.ascontiguousarray(sneg.astype(bf))

    p = np.arange(128)[:, None]
    c = np.arange(128)[None, :]
    triud = np.ascontiguousarray((p <= c).astype(bf))

    per_b = []
    for b in range(B):
        xb = x[b].astype(bf)
        xb = xb.reshape(S, NDT, 128).transpose(1, 0, 2)
        per_b.append(np.ascontiguousarray(xb))

    per_g = []
    for g in range(TPG):
        wq = perm_qk(Wq, g)
        wk = perm_qk(Wk, g)
        wvt_ = tile128(Wv[g * LD:(g + 1) * LD].T)
        wot_ = np.ascontiguousarray(
            Wo[:, g * LD:(g + 1) * LD].T.reshape(2, 128, D).astype(bf))
        per_g.append((wq, wk, wvt_, wot_))

    in_maps = []
    for core in range(NCORES):
        b, g = divmod(core, TPG)
        wq, wk, wvt_, wot_ = per_g[g]
        in_maps.append({
            "xbf": per_b[b],
            "wqt": wq, "wkt": wk, "wvt": wvt_, "wot": wot_,
            "cosd": cosd, "sind": sind, "triud": triud,
        })
    return in_maps


def run(inputs, trace=False):
    """Run on all 8 cores; returns (y_full, BassKernelResults)."""
    x = inputs["x"]
    in_maps = _host_prep(x, inputs["Wq"], inputs["Wk"], inputs["Wv"],
                         inputs["Wo"])
    nc = _get_nc()
    kw = {}
    if trace:
        kw = dict(trace=True, trace_cores=[0])
    res = run_bass_kernel_spmd(nc, in_maps, core_ids=list(range(NCORES)), **kw)
    y = np.zeros((B, S, D), dtype=np.float32)
    for c in range(NCORES):
        y[c // TPG] += res.results[c]["y"]
    return y, res


def kernel(x, Wq, Wk, Wv, Wo, n_heads):
    assert int(n_heads) == H
    y, _ = run({"x": x, "Wq": Wq, "Wk": Wk, "Wv": Wv, "Wo": Wo})
    return y
